# revision 4
# baseline (speedup 1.0000x reference)
"""MemNN layer kernel for 8 Trainium2 NeuronCores.

Strategy (batch-sharded, 16 batches/core):
- The 4 embedding tables are interleaved into one "megatable" whose row v is
  [A0hi|A0lo|A1hi|A1lo|A2hi|A2lo|A3hi|A3lo] (8 x 128 fp16 = 2048 B), where
  hi = fp16(A), lo = fp16(A - hi).  One dma_gather fetches all 4 tables for a
  token at f32-exact precision (hi+lo), at the same GpSimd descriptor-gen cost
  as a single-table gather (cost is per-index, not per-byte).
- dma_gather in transpose mode lands embeddings with embd on partitions:
  G[p, slice, i] = megarow(idx_i)[slice*128+p].  int16 gather indices only
  reach 32767, so two passes per chunk: lo pass (rows < 32768, sentinel row 0
  which is all-zero padding) and hi pass (rows >= 32768 rebased, sentinel ->
  appended all-zero row 50000).
- 20-token sentence sums: DVE tensor_reduce over a 5-D AP that folds the
  lo/hi passes and the 20 tokens in one op -> S[128e, 8slices, 800slots] f32.
- Hops run in embd-on-partition layout: logits via elementwise mul + ones-
  matmul partition reduce, softmax on [1, 800], p broadcast via e0-selector
  matmul, weighted c-sum via DVE reduce.
- Final projection out[v, b] = sum_e A3[v, e] u[e, b]: A3^T fp16 is streamed
  from DRAM (pre-transposed on host) as PE lhsT per 128-vocab chunk, rhs =
  fp16(u); PSUM -> SBUF -> one contiguous store in [128, 391, 16] layout that
  the host rearranges.
"""

import numpy as np

HOPS = 3
VOCAB = 50000
EMBD = 128
BS = 128
STORY = 50
SENT = 20
QLEN = 20
NCORES = 8
BSH = BS // NCORES          # 16 batches per core
SLOTS = BSH * STORY         # 800 (b, s) slots per core
SPLIT = 32768               # int16-reachable rows per gather base
ZROW = VOCAB                # appended all-zero megatable row
HI_SENT = ZROW - SPLIT      # 17232: hi-pass sentinel (-> zero row)
NROWS = VOCAB + 1           # 50001
CHUNKS = [1280] * 12 + [640]  # gather chunks; each %128==0 and %20==0
QPAD = 24                   # per-batch query tokens padded 20 -> 24
QIDX = BSH * QPAD           # 384 (%128 == 0)
VPAD = 50048                # vocab padded to 391*128 for projection
NVC = VPAD // 128           # 391 projection chunks

_cache = {}


def _wrap_idxs(lst):
    """int16 gather index layout: [128, n/16]; position i -> [i%16, i//16], tiled 8x."""
    a = np.asarray(lst).astype(np.int16).reshape(-1, 16).T.copy()
    return np.tile(a, (8, 1))


def _mk_ap(base_ap, dims, extra_offset_elems=0):
    """AP with the partition pair of base_ap and given free (stride, count) pairs."""
    import concourse.bass as bass
    ap = [tuple(base_ap.ap[0])] + [tuple(d) for d in dims]
    return bass.AP(base_ap.tensor, base_ap.offset + extra_offset_elems, ap)


def _build():
    import concourse.tile as tile
    from concourse import bacc, mybir

    f32 = mybir.dt.float32
    f16 = mybir.dt.float16
    i16 = mybir.dt.int16

    nc = bacc.Bacc("TRN2", target_bir_lowering=False, debug=False)

    mega = nc.dram_tensor("mega", [NROWS, 1024], f16, kind="ExternalInput")
    a3t = nc.dram_tensor("a3t", [128, VPAD], f16, kind="ExternalInput")
    ilo = nc.dram_tensor("ilo", [128, SLOTS * SENT // 16], i16, kind="ExternalInput")
    ihi = nc.dram_tensor("ihi", [128, SLOTS * SENT // 16], i16, kind="ExternalInput")
    iqlo = nc.dram_tensor("iqlo", [128, QIDX // 16], i16, kind="ExternalInput")
    iqhi = nc.dram_tensor("iqhi", [128, QIDX // 16], i16, kind="ExternalInput")
    tat = nc.dram_tensor("tat", [128, STORY], f32, kind="ExternalInput")
    tct = nc.dram_tensor("tct", [128, STORY], f32, kind="ExternalInput")
    out = nc.dram_tensor("outp", [128, NVC, BSH], f32, kind="ExternalOutput")

    with tile.TileContext(nc) as tc:
        with (
            tc.tile_pool(name="consts", bufs=1) as cpool,
            tc.tile_pool(name="sacc", bufs=1) as spool,
        ):
            # ---- constants / small loads
            t_tat = cpool.tile([128, STORY], f32, tag="tat")
            nc.sync.dma_start(t_tat[:], tat[:])
            t_tct = cpool.tile([128, STORY], f32, tag="tct")
            nc.sync.dma_start(t_tct[:], tct[:])
            ones_col = cpool.tile([128, 1], f32, tag="ones_col")  # lhsT for partition sum
            nc.vector.memset(ones_col[:], 1.0)
            e0row = cpool.tile([128, 128], f32, tag="e0row")      # lhsT for p broadcast
            nc.vector.memset(e0row[:], 0.0)
            nc.vector.memset(e0row[0:1, :], 1.0)

            t_ilo = cpool.tile([128, SLOTS * SENT // 16], i16, tag="ilo")
            nc.sync.dma_start(t_ilo[:], ilo[:])
            t_ihi = cpool.tile([128, SLOTS * SENT // 16], i16, tag="ihi")
            nc.sync.dma_start(t_ihi[:], ihi[:])
            t_iqlo = cpool.tile([128, QIDX // 16], i16, tag="iqlo")
            nc.sync.dma_start(t_iqlo[:], iqlo[:])
            t_iqhi = cpool.tile([128, QIDX // 16], i16, tag="iqhi")
            nc.sync.dma_start(t_iqhi[:], iqhi[:])

            # ---- S accumulator [128, 8 slices, 800 slots] f32
            S = spool.tile([128, 8, SLOTS], f32, tag="S")
            uq = spool.tile([128, 2, 8, BSH], f32, tag="uq")  # query-sum, per pass+slice

            # ---- gather + reduce phase
            with tc.tile_pool(name="gath", bufs=2) as gpool:
                pos = 0
                for ch in CHUNKS:
                    g = gpool.tile([128, 2, 8, ch], f16, tag="g")
                    cs = slice(pos // 16, (pos + ch) // 16)
                    nc.gpsimd.dma_gather(
                        g[:, 0], mega[:], t_ilo[:, cs], ch, ch, 1024,
                        transpose=True, single_packet=False)
                    nc.gpsimd.dma_gather(
                        g[:, 1], mega[SPLIT:, :], t_ihi[:, cs], ch, ch, 1024,
                        transpose=True, single_packet=False)
                    # reduce over (pass, token): [128, 8, nslot, 2, 20] -> [128, 8, nslot]
                    gap = g[:]
                    red_in = _mk_ap(gap, [(ch, 8), (SENT, ch // SENT), (8 * ch, 2), (1, SENT)])
                    nslot = ch // SENT
                    s0 = pos // SENT
                    nc.vector.tensor_reduce(
                        S[:, :, s0:s0 + nslot], red_in,
                        mybir.AxisListType.XY, mybir.AluOpType.add)
                    pos += ch

                # query-token sums (table 0 slices only are used later)
                gq = gpool.tile([128, 2, 8, QIDX], f16, tag="gq")
                nc.gpsimd.dma_gather(
                    gq[:, 0], mega[:], t_iqlo[:], QIDX, QIDX, 1024,
                    transpose=True, single_packet=False)
                nc.gpsimd.dma_gather(
                    gq[:, 1], mega[SPLIT:, :], t_iqhi[:], QIDX, QIDX, 1024,
                    transpose=True, single_packet=False)
                gqap = gq[:]
                # [128, 2, 8, 16, 24] reduce X(24) -> [128, 2, 8, 16]
                q_in = _mk_ap(gqap, [(8 * QIDX, 2), (QIDX, 8), (QPAD, BSH), (1, QPAD)])
                nc.vector.tensor_reduce(
                    uq[:], q_in, mybir.AxisListType.X, mybir.AluOpType.add)

            with (
                tc.tile_pool(name="hopp", bufs=1) as hpool,
                tc.tile_pool(name="psum", bufs=2, space="PSUM") as ppool,
            ):
                # u0 = sum of q-token embeddings of table 0: hi+lo slices, both passes
                u = hpool.tile([128, BSH], f32, tag="u")
                nc.vector.tensor_add(u[:], uq[:, 0, 0, :], uq[:, 0, 1, :])
                nc.vector.tensor_add(u[:], u[:], uq[:, 1, 0, :])
                nc.vector.tensor_add(u[:], u[:], uq[:, 1, 1, :])

                # SM[k] = S[2k] + S[2k+1]  (fold hi+lo)
                SM = hpool.tile([128, 4, SLOTS], f32, tag="SM")
                for k in range(4):
                    nc.vector.tensor_add(SM[:, k, :], S[:, 2 * k, :], S[:, 2 * k + 1, :])

                t0 = hpool.tile([128, BSH, STORY], f32, tag="t0")
                pe_sb = hpool.tile([128, BSH, STORY], f32, tag="pe_sb")
                nc.vector.memset(pe_sb[:], 0.0)
                lg = hpool.tile([1, BSH, STORY], f32, tag="lg")
                red = hpool.tile([1, BSH], f32, tag="red")
                red2 = hpool.tile([1, BSH], f32, tag="red2")
                red_u = hpool.tile([128, BSH], f32, tag="redu")

                def smv(k, off=0, nb=BSH):
                    return _mk_ap(SM[:], [(STORY, nb), (1, STORY)], k * SLOTS + off * STORY)

                def t0v(off=0, nb=BSH):
                    return _mk_ap(t0[:], [(STORY, nb), (1, STORY)], off * STORY)

                def t0f(off, n):
                    return _mk_ap(t0[:], [(1, n)], off)

                ta_b = _mk_ap(t_tat[:], [(0, BSH), (1, STORY)])
                tc_bh = _mk_ap(t_tct[:], [(0, BSH // 2), (1, STORY)])
                u_b = _mk_ap(u[:], [(1, BSH), (0, STORY)])
                HB = SLOTS // 2  # 400

                for k in range(HOPS):
                    # t0 = (SM[k] + TA bcast) * u bcast
                    nc.vector.tensor_add(t0v(), smv(k), ta_b)
                    nc.vector.tensor_mul(t0v(), t0v(), u_b)
                    # partition-reduce -> logits [1, 16, 50] (two 400-wide psum banks)
                    for h in range(2):
                        pl = ppool.tile([1, HB], f32, tag="pl", space="PSUM")
                        nc.tensor.matmul(
                            pl[:], lhsT=ones_col[:], rhs=t0f(h * HB, HB),
                            start=True, stop=True)
                        nc.vector.tensor_copy(
                            _mk_ap(lg[:], [(1, HB)], h * HB), pl[:])
                    # softmax over story per batch, on partition 0
                    nc.vector.tensor_reduce(red[:], lg[:], mybir.AxisListType.X, mybir.AluOpType.max)
                    red_b = _mk_ap(red[:], [(1, BSH), (0, STORY)])
                    nc.vector.tensor_sub(lg[:], lg[:], red_b)
                    nc.scalar.activation(lg[:], lg[:], mybir.ActivationFunctionType.Exp)
                    nc.vector.tensor_reduce(red2[:], lg[:], mybir.AxisListType.X, mybir.AluOpType.add)
                    nc.vector.reciprocal(red2[:], red2[:])
                    red2_b = _mk_ap(red2[:], [(1, BSH), (0, STORY)])
                    nc.vector.tensor_mul(pe_sb[0:1, :, :], lg[:], red2_b)
                    # broadcast p to all partitions; then t0 = (SM[k+1] + TC bcast) * p
                    for h in range(2):
                        pb = ppool.tile([128, HB], f32, tag="pb", space="PSUM")
                        nc.tensor.matmul(
                            pb[:], lhsT=e0row[:],
                            rhs=_mk_ap(pe_sb[:], [(1, HB)], h * HB),
                            start=True, stop=True)
                        pb3 = _mk_ap(pb[:], [(STORY, BSH // 2), (1, STORY)])
                        nc.vector.tensor_add(
                            t0v(h * (BSH // 2), BSH // 2),
                            smv(k + 1, h * (BSH // 2), BSH // 2), tc_bh)
                        nc.vector.tensor_mul(
                            t0v(h * (BSH // 2), BSH // 2),
                            t0v(h * (BSH // 2), BSH // 2), pb3)
                    # u += sum_s t0
                    nc.vector.tensor_reduce(red_u[:], t0v(), mybir.AxisListType.X, mybir.AluOpType.add)
                    nc.vector.tensor_add(u[:], u[:], red_u[:])

                # ---- projection: out[v, b] = sum_e A3[v, e] * u[e, b]
                u16 = hpool.tile([128, BSH], f16, tag="u16")
                nc.vector.tensor_copy(u16[:], u[:])
                ocache = hpool.tile([128, NVC, BSH], f32, tag="ocache")
                with tc.tile_pool(name="a3pool", bufs=3) as apool:
                    CPL = 16  # vocab chunks (of 128) per a3t load
                    for blk in range(NVC // CPL + (1 if NVC % CPL else 0)):
                        n_in_blk = min(CPL, NVC - blk * CPL)
                        a3c = apool.tile([128, CPL * 128], f16, tag="a3c")
                        nc.sync.dma_start(
                            a3c[:, :n_in_blk * 128],
                            a3t[:, blk * CPL * 128: blk * CPL * 128 + n_in_blk * 128])
                        for j in range(0, n_in_blk, 8):
                            nj = min(8, n_in_blk - j)
                            po = ppool.tile([128, 8 * BSH], f32, tag="po", space="PSUM")
                            for w in range(nj):
                                nc.tensor.matmul(
                                    po[:, w * BSH:(w + 1) * BSH],
                                    lhsT=a3c[:, (j + w) * 128:(j + w + 1) * 128],
                                    rhs=u16[:], start=True, stop=True)
                            c0 = blk * CPL + j
                            nc.vector.tensor_copy(
                                ocache[:, c0:c0 + nj, :], po[:, :nj * BSH])
                nc.sync.dma_start(out[:], ocache[:])

    nc.compile()
    return nc


def _prep_inputs(x, q, A, TA, TC):
    """Host-side marshalling: megatable, A3^T, per-core index lists."""
    x = np.asarray(x).astype(np.int64)
    q = np.asarray(q).astype(np.int64)
    A = np.asarray(A, dtype=np.float32)
    TA = np.asarray(TA, dtype=np.float32)
    TC = np.asarray(TC, dtype=np.float32)

    Ahi = A.astype(np.float16)
    Alo = (A - Ahi.astype(np.float32)).astype(np.float16)
    # megarow v: [A0hi, A0lo, A1hi, A1lo, A2hi, A2lo, A3hi, A3lo]
    mega = np.zeros((NROWS, 1024), dtype=np.float16)
    for k in range(4):
        mega[:VOCAB, (2 * k) * 128:(2 * k) * 128 + 128] = Ahi[k]
        mega[:VOCAB, (2 * k + 1) * 128:(2 * k + 1) * 128 + 128] = Alo[k]
    # row ZROW stays zero (hi-pass sentinel target); row 0 is zero already (padding row)

    a3t = np.zeros((128, VPAD), dtype=np.float16)
    a3t[:, :VOCAB] = Ahi[3].T

    tat = np.ascontiguousarray(TA[0].T)  # [128, 50]
    tct = np.ascontiguousarray(TC[0].T)

    in_maps = []
    for c in range(NCORES):
        xs = x[c * BSH:(c + 1) * BSH].reshape(-1)        # [16000] slot-major
        lo = np.where(xs < SPLIT, xs, 0)
        hi = np.where(xs >= SPLIT, xs - SPLIT, HI_SENT)
        qs = q[c * BSH:(c + 1) * BSH]                     # [16, 20]
        qp = np.full((BSH, QPAD), -1, dtype=np.int64)
        qp[:, :QLEN] = qs
        qf = qp.reshape(-1)
        qlo = np.where((qf >= 0) & (qf < SPLIT), qf, 0)
        qhi = np.where(qf >= SPLIT, qf - SPLIT, HI_SENT)
        in_maps.append({
            "mega": mega, "a3t": a3t, "tat": tat, "tct": tct,
            "ilo": _wrap_idxs(lo), "ihi": _wrap_idxs(hi),
            "iqlo": _wrap_idxs(qlo), "iqhi": _wrap_idxs(qhi),
        })
    return in_maps


def kernel(x, q, A, TA, TC):
    import os
    from concourse.bass_utils import run_bass_kernel_spmd

    if "nc" not in _cache:
        _cache["nc"] = _build()
    nc = _cache["nc"]

    in_maps = _prep_inputs(x, q, A, TA, TC)
    trace = bool(int(os.environ.get("MEMNN_TRACE", "0")))
    res = run_bass_kernel_spmd(nc, in_maps, list(range(NCORES)), trace=trace)
    if trace:
        _cache["exec_time_ns"] = res.exec_time_ns
        _cache["mean_exec_time_ns"] = res.mean_exec_time_ns
        _cache["results"] = res

    outs = []
    for c in range(NCORES):
        oc = res.results[c]["outp"]                       # [128, 391, 16]
        full = oc.transpose(1, 0, 2).reshape(VPAD, BSH)   # [50048, 16]
        outs.append(full[:VOCAB].T)                       # [16, 50000]
    return np.concatenate(outs, axis=0).astype(np.float32)


# revision 13
# speedup vs baseline: 1.7238x; 1.7238x over previous
"""MemNN layer kernel for 8 Trainium2 NeuronCores.

Strategy (batch-sharded, 16 batches/core):
- The 4 embedding tables are interleaved into one "megatable" whose row v is
  [A0hi|A0lo|A1hi|A1lo|A2hi|A2lo|A3hi|A3lo] (8 x 128 fp16 = 2048 B), where
  hi = fp16(A), lo = fp16(A - hi).  One dma_gather fetches all 4 tables for a
  token at f32-exact precision (hi+lo), at the same GpSimd descriptor-gen cost
  as a single-table gather (cost is per-index, not per-byte).
- dma_gather in transpose mode lands embeddings with embd on partitions:
  G[p, slice, i] = megarow(idx_i)[slice*128+p].  int16 gather indices only
  reach 32767, so two passes per chunk: lo pass (rows < 32768, sentinel row 0
  which is all-zero padding) and hi pass (rows >= 32768 rebased, sentinel ->
  appended all-zero row 50000).
- 20-token sentence sums: DVE tensor_reduce over a 5-D AP that folds the
  lo/hi passes and the 20 tokens in one op -> S[128e, 8slices, 800slots] f32.
- Hops run in embd-on-partition layout: logits via elementwise mul + ones-
  matmul partition reduce, softmax on [1, 800], p broadcast via e0-selector
  matmul, weighted c-sum via DVE reduce.
- Final projection out[v, b] = sum_e A3[v, e] u[e, b]: A3^T fp16 is streamed
  from DRAM (pre-transposed on host) as PE lhsT per 128-vocab chunk, rhs =
  fp16(u); PSUM -> SBUF -> one contiguous store in [128, 391, 16] layout that
  the host rearranges.
"""

import numpy as np

HOPS = 3
VOCAB = 50000
EMBD = 128
BS = 128
STORY = 50
SENT = 20
QLEN = 20
NCORES = 8
BSH = BS // NCORES          # 16 batches per core
SLOTS = BSH * STORY         # 800 (b, s) slots per core
SPLIT = 32768               # int16-reachable rows per gather base
ZROW = VOCAB                # appended all-zero megatable row
HI_SENT = ZROW - SPLIT      # 17232: hi-pass sentinel (-> zero row)
NROWS = VOCAB + 1           # 50001
CHUNKS = [2560] * 6 + [640]  # gather chunks; each %128==0 and %20==0
QPAD = 24                   # per-batch query tokens padded 20 -> 24
QIDX = BSH * QPAD           # 384 (%128 == 0)
VPAD = 50048                # vocab padded to 391*128 for projection
NVC = VPAD // 128           # 391 projection chunks

_cache = {}


def _wrap_idxs(lst):
    """int16 gather index layout: [128, n/16]; position i -> [i%16, i//16], tiled 8x."""
    a = np.asarray(lst).astype(np.int16).reshape(-1, 16).T.copy()
    return np.tile(a, (8, 1))


def _mk_ap(base_ap, dims, extra_offset_elems=0):
    """AP with the partition pair of base_ap and given free (stride, count) pairs."""
    import concourse.bass as bass
    ap = [tuple(base_ap.ap[0])] + [tuple(d) for d in dims]
    return bass.AP(base_ap.tensor, base_ap.offset + extra_offset_elems, ap)


def _build(scale, qmap=None):
    import concourse.tile as tile
    from concourse import bacc, mybir

    f32 = mybir.dt.float32
    i32 = mybir.dt.int32
    f16 = mybir.dt.float16
    i16 = mybir.dt.int16

    nc = bacc.Bacc("TRN2", target_bir_lowering=False, debug=False,
                   dynamic_dma_scratch_size=32768)
    gather_names = []
    _gi = [0]

    def _gq():
        i = _gi[0]
        _gi[0] += 1
        return qmap.get(i, 0) if qmap else 0

    mega = nc.dram_tensor("mega", [NROWS, 512], i16, kind="ExternalInput")
    a3t = nc.dram_tensor("a3t", [128, VPAD], f16, kind="ExternalInput")
    ilo = nc.dram_tensor("ilo", [128, SLOTS * SENT // 16], i16, kind="ExternalInput")
    ihi = nc.dram_tensor("ihi", [128, SLOTS * SENT // 16], i16, kind="ExternalInput")
    iqlo = nc.dram_tensor("iqlo", [128, QIDX // 16], i16, kind="ExternalInput")
    iqhi = nc.dram_tensor("iqhi", [128, QIDX // 16], i16, kind="ExternalInput")
    tat = nc.dram_tensor("tat", [128, STORY], f32, kind="ExternalInput")
    tct = nc.dram_tensor("tct", [128, STORY], f32, kind="ExternalInput")
    out = nc.dram_tensor("outp", [128, NVC, BSH], f32, kind="ExternalOutput")

    with tile.TileContext(nc) as tc:
        with (
            tc.tile_pool(name="consts", bufs=1) as cpool,
            tc.tile_pool(name="sacc", bufs=1) as spool,
        ):
            # ---- constants / small loads
            t_tat = cpool.tile([128, STORY], f32, tag="tat")
            nc.sync.dma_start(t_tat[:], tat[:])
            t_tct = cpool.tile([128, STORY], f32, tag="tct")
            nc.sync.dma_start(t_tct[:], tct[:])
            ones_col = cpool.tile([128, 1], f32, tag="ones_col")  # lhsT for partition sum
            nc.vector.memset(ones_col[:], 1.0)
            e0row = cpool.tile([128, 128], f32, tag="e0row")      # lhsT for p broadcast
            nc.vector.memset(e0row[:], 0.0)
            nc.vector.memset(e0row[0:1, :], 1.0)

            t_ilo = cpool.tile([128, SLOTS * SENT // 16], i16, tag="ilo")
            nc.sync.dma_start(t_ilo[:], ilo[:])
            t_ihi = cpool.tile([128, SLOTS * SENT // 16], i16, tag="ihi")
            nc.sync.dma_start(t_ihi[:], ihi[:])
            t_iqlo = cpool.tile([128, QIDX // 16], i16, tag="iqlo")
            nc.sync.dma_start(t_iqlo[:], iqlo[:])
            t_iqhi = cpool.tile([128, QIDX // 16], i16, tag="iqhi")
            nc.sync.dma_start(t_iqhi[:], iqhi[:])

            # ---- S accumulator [128, 8 slices, 800 slots] f32
            S = spool.tile([128, 4, SLOTS], i32, tag="S")
            uq = spool.tile([128, 2, 4, BSH], f32, tag="uq")  # query-sum, per pass+slice

            # ---- gather + reduce phase (int16 megatable, 1024 B rows)
            # int32 accumulation of int16 values is exact; scale applied later
            low_prec = nc.allow_low_precision(reason="int32 accumulation of int16 is exact")
            low_prec.__enter__()
            with tc.tile_pool(name="gath", bufs=3) as gpool:
                pos = 0
                for ci, ch in enumerate(CHUNKS):
                    nslot = ch // SENT
                    s0 = pos // SENT
                    cs = slice(pos // 16, (pos + ch) // 16)
                    g_lo = gpool.tile([128, 4, ch], i16, tag="g_lo")
                    g_hi = gpool.tile([128, 4, ch], i16, tag="g_hi")
                    gather_names.append(nc.gpsimd.dma_gather(
                        g_lo[:], mega[:], t_ilo[:, cs], ch, ch, 512,
                        transpose=True, single_packet=False, queue_num=_gq()).ins.name)
                    gather_names.append(nc.gpsimd.dma_gather(
                        g_hi[:], mega[SPLIT:, :], t_ihi[:, cs], ch, ch, 512,
                        transpose=True, single_packet=False, queue_num=_gq()).ins.name)
                    # DVE reduce over tokens [128, 4, nslot, 20] -> f32
                    hr = gpool.tile([128, 4, ch // SENT], i32, tag="hr")
                    red_lo = _mk_ap(g_lo[:], [(ch, 4), (SENT, nslot), (1, SENT)])
                    red_hi = _mk_ap(g_hi[:], [(ch, 4), (SENT, nslot), (1, SENT)])
                    nc.vector.tensor_reduce(
                        S[:, :, s0:s0 + nslot], red_lo,
                        mybir.AxisListType.X, mybir.AluOpType.add)
                    nc.vector.tensor_reduce(
                        hr[:], red_hi, mybir.AxisListType.X, mybir.AluOpType.add)
                    nc.vector.tensor_add(
                        S[:, :, s0:s0 + nslot], S[:, :, s0:s0 + nslot], hr[:])
                    pos += ch

                # query-token sums (table 0 slices only are used later)
                gq_lo = gpool.tile([128, 4, QIDX], i16, tag="gq_lo")
                gq_hi = gpool.tile([128, 4, QIDX], i16, tag="gq_hi")
                gather_names.append(nc.gpsimd.dma_gather(
                    gq_lo[:], mega[:], t_iqlo[:], QIDX, QIDX, 512,
                    transpose=True, single_packet=False, queue_num=_gq()).ins.name)
                gather_names.append(nc.gpsimd.dma_gather(
                    gq_hi[:], mega[SPLIT:, :], t_iqhi[:], QIDX, QIDX, 512,
                    transpose=True, single_packet=False, queue_num=_gq()).ins.name)
                # [128, 2(pass), 4, 16, 24] reduce X(24) -> uq [128, 2, 4, 16]
                q_lo_in = _mk_ap(gq_lo[:], [(QIDX, 4), (QPAD, BSH), (1, QPAD)])
                q_hi_in = _mk_ap(gq_hi[:], [(QIDX, 4), (QPAD, BSH), (1, QPAD)])
                nc.vector.tensor_reduce(
                    uq[:, 0], q_lo_in, mybir.AxisListType.X, mybir.AluOpType.add)
                nc.vector.tensor_reduce(
                    uq[:, 1], q_hi_in, mybir.AxisListType.X, mybir.AluOpType.add)

            low_prec.__exit__(None, None, None)
            with (
                tc.tile_pool(name="hopp", bufs=1) as hpool,
                tc.tile_pool(name="psum", bufs=2, space="PSUM") as ppool,
            ):
                # u0 = scale * (q-sum table 0, both passes)
                u = hpool.tile([128, BSH], f32, tag="u")
                nc.vector.scalar_tensor_tensor(
                    u[:], uq[:, 0, 0, :], 1.0, uq[:, 1, 0, :],
                    mybir.AluOpType.mult, mybir.AluOpType.add)
                nc.vector.tensor_scalar_mul(u[:], u[:], scale)

                t0 = hpool.tile([128, BSH, STORY], f32, tag="t0")
                pe_sb = hpool.tile([128, BSH, STORY], f32, tag="pe_sb")
                nc.vector.memset(pe_sb[:], 0.0)
                lg = hpool.tile([1, BSH, STORY], f32, tag="lg")
                red = hpool.tile([1, BSH], f32, tag="red")
                red2 = hpool.tile([1, BSH], f32, tag="red2")
                red_u = hpool.tile([128, BSH], f32, tag="redu")

                def smv(k, off=0, nb=BSH):
                    return _mk_ap(S[:], [(STORY, nb), (1, STORY)], k * SLOTS + off * STORY)

                def t0v(off=0, nb=BSH):
                    return _mk_ap(t0[:], [(STORY, nb), (1, STORY)], off * STORY)

                def t0f(off, n):
                    return _mk_ap(t0[:], [(1, n)], off)

                ta_b = _mk_ap(t_tat[:], [(0, BSH), (1, STORY)])
                tc_bh = _mk_ap(t_tct[:], [(0, BSH // 2), (1, STORY)])
                u_b = _mk_ap(u[:], [(1, BSH), (0, STORY)])
                HB = SLOTS // 2  # 400

                for k in range(HOPS):
                    # t0 = (scale * S[k] + TA bcast) * u bcast
                    nc.vector.scalar_tensor_tensor(
                        t0v(), smv(k), scale, ta_b,
                        mybir.AluOpType.mult, mybir.AluOpType.add)
                    nc.vector.tensor_mul(t0v(), t0v(), u_b)
                    # partition-reduce -> logits [1, 16, 50] (two 400-wide psum banks)
                    for h in range(2):
                        pl = ppool.tile([1, HB], f32, tag="pl", space="PSUM")
                        nc.tensor.matmul(
                            pl[:], lhsT=ones_col[:], rhs=t0f(h * HB, HB),
                            start=True, stop=True)
                        nc.vector.tensor_copy(
                            _mk_ap(lg[:], [(1, HB)], h * HB), pl[:])
                    # softmax over story per batch, on partition 0
                    nc.vector.tensor_reduce(red[:], lg[:], mybir.AxisListType.X, mybir.AluOpType.max)
                    red_b = _mk_ap(red[:], [(1, BSH), (0, STORY)])
                    nc.vector.tensor_sub(lg[:], lg[:], red_b)
                    nc.scalar.activation(lg[:], lg[:], mybir.ActivationFunctionType.Exp)
                    nc.vector.tensor_reduce(red2[:], lg[:], mybir.AxisListType.X, mybir.AluOpType.add)
                    nc.vector.reciprocal(red2[:], red2[:])
                    red2_b = _mk_ap(red2[:], [(1, BSH), (0, STORY)])
                    nc.vector.tensor_mul(pe_sb[0:1, :, :], lg[:], red2_b)
                    # broadcast p to all partitions; then t0 = (SM[k+1] + TC bcast) * p
                    for h in range(2):
                        pb = ppool.tile([128, HB], f32, tag="pb", space="PSUM")
                        nc.tensor.matmul(
                            pb[:], lhsT=e0row[:],
                            rhs=_mk_ap(pe_sb[:], [(1, HB)], h * HB),
                            start=True, stop=True)
                        pb3 = _mk_ap(pb[:], [(STORY, BSH // 2), (1, STORY)])
                        nc.vector.scalar_tensor_tensor(
                            t0v(h * (BSH // 2), BSH // 2),
                            smv(k + 1, h * (BSH // 2), BSH // 2), scale, tc_bh,
                            mybir.AluOpType.mult, mybir.AluOpType.add)
                        nc.vector.tensor_mul(
                            t0v(h * (BSH // 2), BSH // 2),
                            t0v(h * (BSH // 2), BSH // 2), pb3)
                    # u += sum_s t0
                    nc.vector.tensor_reduce(red_u[:], t0v(), mybir.AxisListType.X, mybir.AluOpType.add)
                    nc.vector.tensor_add(u[:], u[:], red_u[:])

                # ---- projection: out[v, b] = sum_e A3[v, e] * u[e, b]
                u16 = hpool.tile([128, BSH], f16, tag="u16")
                nc.vector.tensor_copy(u16[:], u[:])
                ocache = hpool.tile([128, NVC, BSH], f32, tag="ocache")
                with tc.tile_pool(name="a3pool", bufs=3) as apool:
                    CPL = 16  # vocab chunks (of 128) per a3t load
                    for blk in range(NVC // CPL + (1 if NVC % CPL else 0)):
                        n_in_blk = min(CPL, NVC - blk * CPL)
                        a3c = apool.tile([128, CPL * 128], f16, tag="a3c")
                        nc.sync.dma_start(
                            a3c[:, :n_in_blk * 128],
                            a3t[:, blk * CPL * 128: blk * CPL * 128 + n_in_blk * 128])
                        for j in range(0, n_in_blk, 8):
                            nj = min(8, n_in_blk - j)
                            po = ppool.tile([128, 8 * BSH], f32, tag="po", space="PSUM")
                            for w in range(nj):
                                nc.tensor.matmul(
                                    po[:, w * BSH:(w + 1) * BSH],
                                    lhsT=a3c[:, (j + w) * 128:(j + w + 1) * 128],
                                    rhs=u16[:], start=True, stop=True)
                            c0 = blk * CPL + j
                            nc.vector.tensor_copy(
                                ocache[:, c0:c0 + nj, :], po[:, :nj * BSH])
                nc.sync.dma_start(out[:], ocache[:])

    nc.compile()
    nc._gather_names = gather_names
    return nc


def _build_tuned(scale):
    """Two-pass build: learn scheduled SWDGE order, then align queue_num with
    Tile's DMASW-lane round-robin (lane n%8 must always see queue n%4)."""
    nc0 = _build(scale)
    sched = []
    for b in nc0.main_func.blocks:
        for i in b.instructions:
            if type(i).__name__ == "InstDMAGatherAnt":
                sched.append(i.name)
    return nc0  # multi-queue SWDGE corrupts concurrent transpose gathers on HW


def _prep_inputs(x, q, A, TA, TC):
    """Host-side marshalling: megatable, A3^T, per-core index lists."""
    x = np.asarray(x).astype(np.int64)
    q = np.asarray(q).astype(np.int64)
    A = np.asarray(A, dtype=np.float32)
    TA = np.asarray(TA, dtype=np.float32)
    TC = np.asarray(TC, dtype=np.float32)

    s = float(np.abs(A).max())
    scale = s / 32767.0
    Aq = np.round(A / s * 32767.0).astype(np.int16)
    # megarow v: [A0, A1, A2, A3] int16 (1024 B)
    mega = np.zeros((NROWS, 512), dtype=np.int16)
    for k in range(4):
        mega[:VOCAB, k * 128:(k + 1) * 128] = Aq[k]
    # row ZROW stays zero (hi-pass sentinel target); row 0 is zero already (padding row)

    a3t = np.zeros((128, VPAD), dtype=np.float16)
    a3t[:, :VOCAB] = A[3].astype(np.float16).T

    tat = np.ascontiguousarray(TA[0].T)  # [128, 50]
    tct = np.ascontiguousarray(TC[0].T)

    in_maps = []
    for c in range(NCORES):
        xs = x[c * BSH:(c + 1) * BSH].reshape(-1)        # [16000] slot-major
        lo = np.where(xs < SPLIT, xs, 0)
        hi = np.where(xs >= SPLIT, xs - SPLIT, HI_SENT)
        qs = q[c * BSH:(c + 1) * BSH]                     # [16, 20]
        qp = np.full((BSH, QPAD), -1, dtype=np.int64)
        qp[:, :QLEN] = qs
        qf = qp.reshape(-1)
        qlo = np.where((qf >= 0) & (qf < SPLIT), qf, 0)
        qhi = np.where(qf >= SPLIT, qf - SPLIT, HI_SENT)
        in_maps.append({
            "mega": mega, "a3t": a3t, "tat": tat, "tct": tct,
            "ilo": _wrap_idxs(lo), "ihi": _wrap_idxs(hi),
            "iqlo": _wrap_idxs(qlo), "iqhi": _wrap_idxs(qhi),
        })
    return in_maps, scale


def kernel(x, q, A, TA, TC):
    import os
    from concourse.bass_utils import run_bass_kernel_spmd

    in_maps, scale = _prep_inputs(x, q, A, TA, TC)
    if _cache.get("scale") != scale:
        _cache["nc"] = _build_tuned(scale)
        _cache["scale"] = scale
    nc = _cache["nc"]
    trace = bool(int(os.environ.get("MEMNN_TRACE", "0")))
    res = run_bass_kernel_spmd(nc, in_maps, list(range(NCORES)), trace=trace)
    if trace:
        _cache["exec_time_ns"] = res.exec_time_ns
        _cache["mean_exec_time_ns"] = res.mean_exec_time_ns
        _cache["results"] = res

    outs = []
    for c in range(NCORES):
        oc = res.results[c]["outp"]                       # [128, 391, 16]
        full = oc.transpose(1, 0, 2).reshape(VPAD, BSH)   # [50048, 16]
        outs.append(full[:VOCAB].T)                       # [16, 50000]
    return np.concatenate(outs, axis=0).astype(np.float32)


# revision 14
# speedup vs baseline: 1.8120x; 1.0512x over previous
"""MemNN layer kernel for 8 Trainium2 NeuronCores.

Strategy (batch-sharded, 16 batches/core):
- The 4 embedding tables are interleaved into one "megatable" whose row v is
  [A0hi|A0lo|A1hi|A1lo|A2hi|A2lo|A3hi|A3lo] (8 x 128 fp16 = 2048 B), where
  hi = fp16(A), lo = fp16(A - hi).  One dma_gather fetches all 4 tables for a
  token at f32-exact precision (hi+lo), at the same GpSimd descriptor-gen cost
  as a single-table gather (cost is per-index, not per-byte).
- dma_gather in transpose mode lands embeddings with embd on partitions:
  G[p, slice, i] = megarow(idx_i)[slice*128+p].  int16 gather indices only
  reach 32767, so two passes per chunk: lo pass (rows < 32768, sentinel row 0
  which is all-zero padding) and hi pass (rows >= 32768 rebased, sentinel ->
  appended all-zero row 50000).
- 20-token sentence sums: DVE tensor_reduce over a 5-D AP that folds the
  lo/hi passes and the 20 tokens in one op -> S[128e, 8slices, 800slots] f32.
- Hops run in embd-on-partition layout: logits via elementwise mul + ones-
  matmul partition reduce, softmax on [1, 800], p broadcast via e0-selector
  matmul, weighted c-sum via DVE reduce.
- Final projection out[v, b] = sum_e A3[v, e] u[e, b]: A3^T fp16 is streamed
  from DRAM (pre-transposed on host) as PE lhsT per 128-vocab chunk, rhs =
  fp16(u); PSUM -> SBUF -> one contiguous store in [128, 391, 16] layout that
  the host rearranges.
"""

import numpy as np

HOPS = 3
VOCAB = 50000
EMBD = 128
BS = 128
STORY = 50
SENT = 20
QLEN = 20
NCORES = 8
BSH = BS // NCORES          # 16 batches per core
SLOTS = BSH * STORY         # 800 (b, s) slots per core
SPLIT = 32768               # int16-reachable rows per gather base
ZROW = VOCAB                # appended all-zero megatable row
HI_SENT = ZROW - SPLIT      # 17232: hi-pass sentinel (-> zero row)
NROWS = VOCAB + 1           # 50001
CHUNKS = [1280] * 12 + [640]  # %128==0, %20==0; 322 rx descs/lane so 3 overlap in the 1024 ring
QPAD = 24                   # per-batch query tokens padded 20 -> 24
QIDX = BSH * QPAD           # 384 (%128 == 0)
VPAD = 50048                # vocab padded to 391*128 for projection
NVC = VPAD // 128           # 391 projection chunks

_cache = {}


def _wrap_idxs(lst):
    """int16 gather index layout: [128, n/16]; position i -> [i%16, i//16], tiled 8x."""
    a = np.asarray(lst).astype(np.int16).reshape(-1, 16).T.copy()
    return np.tile(a, (8, 1))


def _mk_ap(base_ap, dims, extra_offset_elems=0):
    """AP with the partition pair of base_ap and given free (stride, count) pairs."""
    import concourse.bass as bass
    ap = [tuple(base_ap.ap[0])] + [tuple(d) for d in dims]
    return bass.AP(base_ap.tensor, base_ap.offset + extra_offset_elems, ap)


def _build(scale, qmap=None):
    import concourse.tile as tile
    from concourse import bacc, mybir

    f32 = mybir.dt.float32
    i32 = mybir.dt.int32
    f16 = mybir.dt.float16
    i16 = mybir.dt.int16

    nc = bacc.Bacc("TRN2", target_bir_lowering=False, debug=False)
    gather_names = []
    _gi = [0]

    def _gq():
        i = _gi[0]
        _gi[0] += 1
        return qmap.get(i, 0) if qmap else 0

    mega = nc.dram_tensor("mega", [NROWS, 512], i16, kind="ExternalInput")
    a3t = nc.dram_tensor("a3t", [128, VPAD], f16, kind="ExternalInput")
    ilo = nc.dram_tensor("ilo", [128, SLOTS * SENT // 16], i16, kind="ExternalInput")
    ihi = nc.dram_tensor("ihi", [128, SLOTS * SENT // 16], i16, kind="ExternalInput")
    iqlo = nc.dram_tensor("iqlo", [128, QIDX // 16], i16, kind="ExternalInput")
    iqhi = nc.dram_tensor("iqhi", [128, QIDX // 16], i16, kind="ExternalInput")
    tat = nc.dram_tensor("tat", [128, STORY], f32, kind="ExternalInput")
    tct = nc.dram_tensor("tct", [128, STORY], f32, kind="ExternalInput")
    out = nc.dram_tensor("outp", [128, NVC, BSH], f32, kind="ExternalOutput")

    with tile.TileContext(nc) as tc:
        with (
            tc.tile_pool(name="consts", bufs=1) as cpool,
            tc.tile_pool(name="sacc", bufs=1) as spool,
        ):
            # ---- constants / small loads
            t_tat = cpool.tile([128, STORY], f32, tag="tat")
            nc.sync.dma_start(t_tat[:], tat[:])
            t_tct = cpool.tile([128, STORY], f32, tag="tct")
            nc.sync.dma_start(t_tct[:], tct[:])
            ones_col = cpool.tile([128, 1], f32, tag="ones_col")  # lhsT for partition sum
            nc.vector.memset(ones_col[:], 1.0)
            e0row = cpool.tile([128, 128], f32, tag="e0row")      # lhsT for p broadcast
            nc.vector.memset(e0row[:], 0.0)
            nc.vector.memset(e0row[0:1, :], 1.0)

            t_ilo = cpool.tile([128, SLOTS * SENT // 16], i16, tag="ilo")
            nc.sync.dma_start(t_ilo[:], ilo[:])
            t_ihi = cpool.tile([128, SLOTS * SENT // 16], i16, tag="ihi")
            nc.sync.dma_start(t_ihi[:], ihi[:])
            t_iqlo = cpool.tile([128, QIDX // 16], i16, tag="iqlo")
            nc.sync.dma_start(t_iqlo[:], iqlo[:])
            t_iqhi = cpool.tile([128, QIDX // 16], i16, tag="iqhi")
            nc.sync.dma_start(t_iqhi[:], iqhi[:])

            # ---- S accumulator [128, 8 slices, 800 slots] f32
            S = spool.tile([128, 4, SLOTS], i32, tag="S")
            uq = spool.tile([128, 2, 4, BSH], f32, tag="uq")  # query-sum, per pass+slice

            # ---- gather + reduce phase (int16 megatable, 1024 B rows)
            # int32 accumulation of int16 values is exact; scale applied later
            low_prec = nc.allow_low_precision(reason="int32 accumulation of int16 is exact")
            low_prec.__enter__()
            with tc.tile_pool(name="gath", bufs=3) as gpool:
                pos = 0
                for ci, ch in enumerate(CHUNKS):
                    nslot = ch // SENT
                    s0 = pos // SENT
                    cs = slice(pos // 16, (pos + ch) // 16)
                    g_lo = gpool.tile([128, 4, ch], i16, tag="g_lo")
                    g_hi = gpool.tile([128, 4, ch], i16, tag="g_hi")
                    gather_names.append(nc.gpsimd.dma_gather(
                        g_lo[:], mega[:], t_ilo[:, cs], ch, ch, 512,
                        transpose=True, single_packet=False, queue_num=_gq()).ins.name)
                    gather_names.append(nc.gpsimd.dma_gather(
                        g_hi[:], mega[SPLIT:, :], t_ihi[:, cs], ch, ch, 512,
                        transpose=True, single_packet=False, queue_num=_gq()).ins.name)
                    # DVE reduce over tokens [128, 4, nslot, 20] -> f32
                    hr = gpool.tile([128, 4, ch // SENT], i32, tag="hr")
                    red_lo = _mk_ap(g_lo[:], [(ch, 4), (SENT, nslot), (1, SENT)])
                    red_hi = _mk_ap(g_hi[:], [(ch, 4), (SENT, nslot), (1, SENT)])
                    nc.vector.tensor_reduce(
                        S[:, :, s0:s0 + nslot], red_lo,
                        mybir.AxisListType.X, mybir.AluOpType.add)
                    nc.vector.tensor_reduce(
                        hr[:], red_hi, mybir.AxisListType.X, mybir.AluOpType.add)
                    nc.vector.tensor_add(
                        S[:, :, s0:s0 + nslot], S[:, :, s0:s0 + nslot], hr[:])
                    pos += ch

                # query-token sums (table 0 slices only are used later)
                gq_lo = gpool.tile([128, 4, QIDX], i16, tag="gq_lo")
                gq_hi = gpool.tile([128, 4, QIDX], i16, tag="gq_hi")
                gather_names.append(nc.gpsimd.dma_gather(
                    gq_lo[:], mega[:], t_iqlo[:], QIDX, QIDX, 512,
                    transpose=True, single_packet=False, queue_num=_gq()).ins.name)
                gather_names.append(nc.gpsimd.dma_gather(
                    gq_hi[:], mega[SPLIT:, :], t_iqhi[:], QIDX, QIDX, 512,
                    transpose=True, single_packet=False, queue_num=_gq()).ins.name)
                # [128, 2(pass), 4, 16, 24] reduce X(24) -> uq [128, 2, 4, 16]
                q_lo_in = _mk_ap(gq_lo[:], [(QIDX, 4), (QPAD, BSH), (1, QPAD)])
                q_hi_in = _mk_ap(gq_hi[:], [(QIDX, 4), (QPAD, BSH), (1, QPAD)])
                nc.vector.tensor_reduce(
                    uq[:, 0], q_lo_in, mybir.AxisListType.X, mybir.AluOpType.add)
                nc.vector.tensor_reduce(
                    uq[:, 1], q_hi_in, mybir.AxisListType.X, mybir.AluOpType.add)

            low_prec.__exit__(None, None, None)
            with (
                tc.tile_pool(name="hopp", bufs=1) as hpool,
                tc.tile_pool(name="psum", bufs=2, space="PSUM") as ppool,
            ):
                # u0 = scale * (q-sum table 0, both passes)
                u = hpool.tile([128, BSH], f32, tag="u")
                nc.vector.scalar_tensor_tensor(
                    u[:], uq[:, 0, 0, :], 1.0, uq[:, 1, 0, :],
                    mybir.AluOpType.mult, mybir.AluOpType.add)
                nc.vector.tensor_scalar_mul(u[:], u[:], scale)

                t0 = hpool.tile([128, BSH, STORY], f32, tag="t0")
                pe_sb = hpool.tile([128, BSH, STORY], f32, tag="pe_sb")
                nc.vector.memset(pe_sb[:], 0.0)
                lg = hpool.tile([1, BSH, STORY], f32, tag="lg")
                red = hpool.tile([1, BSH], f32, tag="red")
                red2 = hpool.tile([1, BSH], f32, tag="red2")
                red_u = hpool.tile([128, BSH], f32, tag="redu")

                def smv(k, off=0, nb=BSH):
                    return _mk_ap(S[:], [(STORY, nb), (1, STORY)], k * SLOTS + off * STORY)

                def t0v(off=0, nb=BSH):
                    return _mk_ap(t0[:], [(STORY, nb), (1, STORY)], off * STORY)

                def t0f(off, n):
                    return _mk_ap(t0[:], [(1, n)], off)

                ta_b = _mk_ap(t_tat[:], [(0, BSH), (1, STORY)])
                tc_bh = _mk_ap(t_tct[:], [(0, BSH // 2), (1, STORY)])
                u_b = _mk_ap(u[:], [(1, BSH), (0, STORY)])
                HB = SLOTS // 2  # 400

                for k in range(HOPS):
                    # t0 = (scale * S[k] + TA bcast) * u bcast
                    nc.vector.scalar_tensor_tensor(
                        t0v(), smv(k), scale, ta_b,
                        mybir.AluOpType.mult, mybir.AluOpType.add)
                    nc.vector.tensor_mul(t0v(), t0v(), u_b)
                    # partition-reduce -> logits [1, 16, 50] (two 400-wide psum banks)
                    for h in range(2):
                        pl = ppool.tile([1, HB], f32, tag="pl", space="PSUM")
                        nc.tensor.matmul(
                            pl[:], lhsT=ones_col[:], rhs=t0f(h * HB, HB),
                            start=True, stop=True)
                        nc.vector.tensor_copy(
                            _mk_ap(lg[:], [(1, HB)], h * HB), pl[:])
                    # softmax over story per batch, on partition 0
                    nc.vector.tensor_reduce(red[:], lg[:], mybir.AxisListType.X, mybir.AluOpType.max)
                    red_b = _mk_ap(red[:], [(1, BSH), (0, STORY)])
                    nc.vector.tensor_sub(lg[:], lg[:], red_b)
                    nc.scalar.activation(lg[:], lg[:], mybir.ActivationFunctionType.Exp)
                    nc.vector.tensor_reduce(red2[:], lg[:], mybir.AxisListType.X, mybir.AluOpType.add)
                    nc.vector.reciprocal(red2[:], red2[:])
                    red2_b = _mk_ap(red2[:], [(1, BSH), (0, STORY)])
                    nc.vector.tensor_mul(pe_sb[0:1, :, :], lg[:], red2_b)
                    # broadcast p to all partitions; then t0 = (SM[k+1] + TC bcast) * p
                    for h in range(2):
                        pb = ppool.tile([128, HB], f32, tag="pb", space="PSUM")
                        nc.tensor.matmul(
                            pb[:], lhsT=e0row[:],
                            rhs=_mk_ap(pe_sb[:], [(1, HB)], h * HB),
                            start=True, stop=True)
                        pb3 = _mk_ap(pb[:], [(STORY, BSH // 2), (1, STORY)])
                        nc.vector.scalar_tensor_tensor(
                            t0v(h * (BSH // 2), BSH // 2),
                            smv(k + 1, h * (BSH // 2), BSH // 2), scale, tc_bh,
                            mybir.AluOpType.mult, mybir.AluOpType.add)
                        nc.vector.tensor_mul(
                            t0v(h * (BSH // 2), BSH // 2),
                            t0v(h * (BSH // 2), BSH // 2), pb3)
                    # u += sum_s t0
                    nc.vector.tensor_reduce(red_u[:], t0v(), mybir.AxisListType.X, mybir.AluOpType.add)
                    nc.vector.tensor_add(u[:], u[:], red_u[:])

                # ---- projection: out[v, b] = sum_e A3[v, e] * u[e, b]
                u16 = hpool.tile([128, BSH], f16, tag="u16")
                nc.vector.tensor_copy(u16[:], u[:])
                ocache = hpool.tile([128, NVC, BSH], f32, tag="ocache")
                with tc.tile_pool(name="a3pool", bufs=3) as apool:
                    CPL = 16  # vocab chunks (of 128) per a3t load
                    for blk in range(NVC // CPL + (1 if NVC % CPL else 0)):
                        n_in_blk = min(CPL, NVC - blk * CPL)
                        a3c = apool.tile([128, CPL * 128], f16, tag="a3c")
                        nc.sync.dma_start(
                            a3c[:, :n_in_blk * 128],
                            a3t[:, blk * CPL * 128: blk * CPL * 128 + n_in_blk * 128])
                        for j in range(0, n_in_blk, 8):
                            nj = min(8, n_in_blk - j)
                            po = ppool.tile([128, 8 * BSH], f32, tag="po", space="PSUM")
                            for w in range(nj):
                                nc.tensor.matmul(
                                    po[:, w * BSH:(w + 1) * BSH],
                                    lhsT=a3c[:, (j + w) * 128:(j + w + 1) * 128],
                                    rhs=u16[:], start=True, stop=True)
                            c0 = blk * CPL + j
                            nc.vector.tensor_copy(
                                ocache[:, c0:c0 + nj, :], po[:, :nj * BSH])
                nc.sync.dma_start(out[:], ocache[:])

    nc.compile()
    nc._gather_names = gather_names
    return nc


def _build_tuned(scale):
    """Two-pass build: learn scheduled SWDGE order, then align queue_num with
    Tile's DMASW-lane round-robin (lane n%8 must always see queue n%4)."""
    nc0 = _build(scale)
    sched = []
    for b in nc0.main_func.blocks:
        for i in b.instructions:
            if type(i).__name__ == "InstDMAGatherAnt":
                sched.append(i.name)
    return nc0  # multi-queue SWDGE corrupts concurrent transpose gathers on HW


def _prep_inputs(x, q, A, TA, TC):
    """Host-side marshalling: megatable, A3^T, per-core index lists."""
    x = np.asarray(x).astype(np.int64)
    q = np.asarray(q).astype(np.int64)
    A = np.asarray(A, dtype=np.float32)
    TA = np.asarray(TA, dtype=np.float32)
    TC = np.asarray(TC, dtype=np.float32)

    s = float(np.abs(A).max())
    scale = s / 32767.0
    Aq = np.round(A / s * 32767.0).astype(np.int16)
    # megarow v: [A0, A1, A2, A3] int16 (1024 B)
    mega = np.zeros((NROWS, 512), dtype=np.int16)
    for k in range(4):
        mega[:VOCAB, k * 128:(k + 1) * 128] = Aq[k]
    # row ZROW stays zero (hi-pass sentinel target); row 0 is zero already (padding row)

    a3t = np.zeros((128, VPAD), dtype=np.float16)
    a3t[:, :VOCAB] = A[3].astype(np.float16).T

    tat = np.ascontiguousarray(TA[0].T)  # [128, 50]
    tct = np.ascontiguousarray(TC[0].T)

    in_maps = []
    for c in range(NCORES):
        xs = x[c * BSH:(c + 1) * BSH].reshape(-1)        # [16000] slot-major
        lo = np.where(xs < SPLIT, xs, 0)
        hi = np.where(xs >= SPLIT, xs - SPLIT, HI_SENT)
        qs = q[c * BSH:(c + 1) * BSH]                     # [16, 20]
        qp = np.full((BSH, QPAD), -1, dtype=np.int64)
        qp[:, :QLEN] = qs
        qf = qp.reshape(-1)
        qlo = np.where((qf >= 0) & (qf < SPLIT), qf, 0)
        qhi = np.where(qf >= SPLIT, qf - SPLIT, HI_SENT)
        in_maps.append({
            "mega": mega, "a3t": a3t, "tat": tat, "tct": tct,
            "ilo": _wrap_idxs(lo), "ihi": _wrap_idxs(hi),
            "iqlo": _wrap_idxs(qlo), "iqhi": _wrap_idxs(qhi),
        })
    return in_maps, scale


def kernel(x, q, A, TA, TC):
    import os
    from concourse.bass_utils import run_bass_kernel_spmd

    in_maps, scale = _prep_inputs(x, q, A, TA, TC)
    if _cache.get("scale") != scale:
        _cache["nc"] = _build_tuned(scale)
        _cache["scale"] = scale
    nc = _cache["nc"]
    trace = bool(int(os.environ.get("MEMNN_TRACE", "0")))
    res = run_bass_kernel_spmd(nc, in_maps, list(range(NCORES)), trace=trace)
    if trace:
        _cache["exec_time_ns"] = res.exec_time_ns
        _cache["mean_exec_time_ns"] = res.mean_exec_time_ns
        _cache["results"] = res

    outs = []
    for c in range(NCORES):
        oc = res.results[c]["outp"]                       # [128, 391, 16]
        full = oc.transpose(1, 0, 2).reshape(VPAD, BSH)   # [50048, 16]
        outs.append(full[:VOCAB].T)                       # [16, 50000]
    return np.concatenate(outs, axis=0).astype(np.float32)


# revision 15
# speedup vs baseline: 1.9185x; 1.0588x over previous
"""MemNN layer kernel for 8 Trainium2 NeuronCores.

Strategy (batch-sharded, 16 batches/core):
- The 4 embedding tables are interleaved into one "megatable" whose row v is
  [A0hi|A0lo|A1hi|A1lo|A2hi|A2lo|A3hi|A3lo] (8 x 128 fp16 = 2048 B), where
  hi = fp16(A), lo = fp16(A - hi).  One dma_gather fetches all 4 tables for a
  token at f32-exact precision (hi+lo), at the same GpSimd descriptor-gen cost
  as a single-table gather (cost is per-index, not per-byte).
- dma_gather in transpose mode lands embeddings with embd on partitions:
  G[p, slice, i] = megarow(idx_i)[slice*128+p].  int16 gather indices only
  reach 32767, so two passes per chunk: lo pass (rows < 32768, sentinel row 0
  which is all-zero padding) and hi pass (rows >= 32768 rebased, sentinel ->
  appended all-zero row 50000).
- 20-token sentence sums: DVE tensor_reduce over a 5-D AP that folds the
  lo/hi passes and the 20 tokens in one op -> S[128e, 8slices, 800slots] f32.
- Hops run in embd-on-partition layout: logits via elementwise mul + ones-
  matmul partition reduce, softmax on [1, 800], p broadcast via e0-selector
  matmul, weighted c-sum via DVE reduce.
- Final projection out[v, b] = sum_e A3[v, e] u[e, b]: A3^T fp16 is streamed
  from DRAM (pre-transposed on host) as PE lhsT per 128-vocab chunk, rhs =
  fp16(u); PSUM -> SBUF -> one contiguous store in [128, 391, 16] layout that
  the host rearranges.
"""

import numpy as np

HOPS = 3
VOCAB = 50000
EMBD = 128
BS = 128
STORY = 50
SENT = 20
QLEN = 20
NCORES = 8
BSH = BS // NCORES          # 16 batches per core
SLOTS = BSH * STORY         # 800 (b, s) slots per core
SPLIT = 32768               # int16-reachable rows per gather base
ZROW = VOCAB                # appended all-zero megatable row
HI_SENT = ZROW - SPLIT      # 17232: hi-pass sentinel (-> zero row)
NROWS = VOCAB + 1           # 50001
CHUNKS = [1280] * 12 + [640]  # %128==0, %20==0; 322 rx descs/lane so 3 overlap in the 1024 ring
QPAD = 24                   # per-batch query tokens padded 20 -> 24
QIDX = BSH * QPAD           # 384 (%128 == 0)
VPAD = 50048                # vocab padded to 391*128 for projection
NVC = VPAD // 128           # 391 projection chunks

_cache = {}


def _wrap_idxs(lst):
    """int16 gather index layout: [128, n/16]; position i -> [i%16, i//16], tiled 8x."""
    a = np.asarray(lst).astype(np.int16).reshape(-1, 16).T.copy()
    return np.tile(a, (8, 1))


def _mk_ap(base_ap, dims, extra_offset_elems=0):
    """AP with the partition pair of base_ap and given free (stride, count) pairs."""
    import concourse.bass as bass
    ap = [tuple(base_ap.ap[0])] + [tuple(d) for d in dims]
    return bass.AP(base_ap.tensor, base_ap.offset + extra_offset_elems, ap)


def _build(scale, qmap=None):
    import concourse.tile as tile
    from concourse import bacc, mybir

    f32 = mybir.dt.float32
    i32 = mybir.dt.int32
    f16 = mybir.dt.float16
    i16 = mybir.dt.int16

    nc = bacc.Bacc("TRN2", target_bir_lowering=False, debug=False)
    gather_names = []
    _gi = [0]

    def _gq():
        i = _gi[0]
        _gi[0] += 1
        return qmap.get(i, 0) if qmap else 0

    mega = nc.dram_tensor("mega", [NROWS, 512], i16, kind="ExternalInput")
    a3t = nc.dram_tensor("a3t", [128, VPAD], f16, kind="ExternalInput")
    ilo = nc.dram_tensor("ilo", [128, SLOTS * SENT // 16], i16, kind="ExternalInput")
    ihi = nc.dram_tensor("ihi", [128, SLOTS * SENT // 16], i16, kind="ExternalInput")
    iqlo = nc.dram_tensor("iqlo", [128, QIDX // 16], i16, kind="ExternalInput")
    iqhi = nc.dram_tensor("iqhi", [128, QIDX // 16], i16, kind="ExternalInput")
    tat = nc.dram_tensor("tat", [128, STORY], f32, kind="ExternalInput")
    tct = nc.dram_tensor("tct", [128, STORY], f32, kind="ExternalInput")
    out = nc.dram_tensor("outp", [128, NVC, BSH], f32, kind="ExternalOutput")

    with tile.TileContext(nc) as tc:
        with (
            tc.tile_pool(name="consts", bufs=1) as cpool,
            tc.tile_pool(name="sacc", bufs=1) as spool,
        ):
            # ---- constants / small loads
            t_tat = cpool.tile([128, STORY], f32, tag="tat")
            nc.sync.dma_start(t_tat[:], tat[:])
            t_tct = cpool.tile([128, STORY], f32, tag="tct")
            nc.sync.dma_start(t_tct[:], tct[:])
            ones_col = cpool.tile([128, 1], f32, tag="ones_col")  # lhsT for partition sum
            nc.vector.memset(ones_col[:], 1.0)
            e0row = cpool.tile([128, 128], f32, tag="e0row")      # lhsT for p broadcast
            nc.vector.memset(e0row[:], 0.0)
            nc.vector.memset(e0row[0:1, :], 1.0)

            t_ilo = cpool.tile([128, SLOTS * SENT // 16], i16, tag="ilo")
            nc.sync.dma_start(t_ilo[:], ilo[:])
            t_ihi = cpool.tile([128, SLOTS * SENT // 16], i16, tag="ihi")
            nc.sync.dma_start(t_ihi[:], ihi[:])
            t_iqlo = cpool.tile([128, QIDX // 16], i16, tag="iqlo")
            nc.sync.dma_start(t_iqlo[:], iqlo[:])
            t_iqhi = cpool.tile([128, QIDX // 16], i16, tag="iqhi")
            nc.sync.dma_start(t_iqhi[:], iqhi[:])

            # ---- S accumulator [128, 8 slices, 800 slots] f32
            S = spool.tile([128, 4, SLOTS], i32, tag="S")
            uq = spool.tile([128, 2, 4, BSH], f32, tag="uq")  # query-sum, per pass+slice

            # ---- gather + reduce phase (int16 megatable, 1024 B rows)
            # int32 accumulation of int16 values is exact; scale applied later
            low_prec = nc.allow_low_precision(reason="int32 accumulation of int16 is exact")
            low_prec.__enter__()
            with tc.tile_pool(name="gath", bufs=3) as gpool:
                pos = 0
                for ci, ch in enumerate(CHUNKS):
                    nslot = ch // SENT
                    s0 = pos // SENT
                    cs = slice(pos // 16, (pos + ch) // 16)
                    g_lo = gpool.tile([128, 4, ch], i16, tag="g_lo")
                    g_hi = gpool.tile([128, 4, ch], i16, tag="g_hi")
                    gather_names.append(nc.gpsimd.dma_gather(
                        g_lo[:], mega[:], t_ilo[:, cs], ch, ch, 512,
                        transpose=True, single_packet=False, queue_num=_gq()).ins.name)
                    gather_names.append(nc.gpsimd.dma_gather(
                        g_hi[:], mega[SPLIT:, :], t_ihi[:, cs], ch, ch, 512,
                        transpose=True, single_packet=False, queue_num=_gq()).ins.name)
                    # DVE reduce over tokens [128, 4, nslot, 20] -> f32
                    hr = gpool.tile([128, 4, ch // SENT], i32, tag="hr")
                    red_lo = _mk_ap(g_lo[:], [(ch, 4), (SENT, nslot), (1, SENT)])
                    red_hi = _mk_ap(g_hi[:], [(ch, 4), (SENT, nslot), (1, SENT)])
                    nc.vector.tensor_reduce(
                        S[:, :, s0:s0 + nslot], red_lo,
                        mybir.AxisListType.X, mybir.AluOpType.add)
                    nc.vector.tensor_reduce(
                        hr[:], red_hi, mybir.AxisListType.X, mybir.AluOpType.add)
                    nc.vector.tensor_add(
                        S[:, :, s0:s0 + nslot], S[:, :, s0:s0 + nslot], hr[:])
                    pos += ch

                # query-token sums (table 0 slices only are used later)
                gq_lo = gpool.tile([128, 4, QIDX], i16, tag="gq_lo")
                gq_hi = gpool.tile([128, 4, QIDX], i16, tag="gq_hi")
                gather_names.append(nc.gpsimd.dma_gather(
                    gq_lo[:], mega[:], t_iqlo[:], QIDX, QIDX, 512,
                    transpose=True, single_packet=False, queue_num=_gq()).ins.name)
                gather_names.append(nc.gpsimd.dma_gather(
                    gq_hi[:], mega[SPLIT:, :], t_iqhi[:], QIDX, QIDX, 512,
                    transpose=True, single_packet=False, queue_num=_gq()).ins.name)
                # [128, 2(pass), 4, 16, 24] reduce X(24) -> uq [128, 2, 4, 16]
                q_lo_in = _mk_ap(gq_lo[:], [(QIDX, 4), (QPAD, BSH), (1, QPAD)])
                q_hi_in = _mk_ap(gq_hi[:], [(QIDX, 4), (QPAD, BSH), (1, QPAD)])
                nc.vector.tensor_reduce(
                    uq[:, 0], q_lo_in, mybir.AxisListType.X, mybir.AluOpType.add)
                nc.vector.tensor_reduce(
                    uq[:, 1], q_hi_in, mybir.AxisListType.X, mybir.AluOpType.add)

            low_prec.__exit__(None, None, None)
            with (
                tc.tile_pool(name="hopp", bufs=1) as hpool,
                tc.tile_pool(name="psum", bufs=2, space="PSUM") as ppool,
            ):
                # u0 = scale * (q-sum table 0, both passes)
                u = hpool.tile([128, BSH], f32, tag="u")
                nc.vector.scalar_tensor_tensor(
                    u[:], uq[:, 0, 0, :], 1.0, uq[:, 1, 0, :],
                    mybir.AluOpType.mult, mybir.AluOpType.add)
                nc.vector.tensor_scalar_mul(u[:], u[:], scale)

                t0 = hpool.tile([128, BSH, STORY], f32, tag="t0")
                pe_sb = hpool.tile([128, BSH, STORY], f32, tag="pe_sb")
                nc.vector.memset(pe_sb[:], 0.0)
                lg = hpool.tile([1, BSH, STORY], f32, tag="lg")
                red = hpool.tile([1, BSH], f32, tag="red")
                red2 = hpool.tile([1, BSH], f32, tag="red2")
                red_u = hpool.tile([128, BSH], f32, tag="redu")

                def smv(k, off=0, nb=BSH):
                    return _mk_ap(S[:], [(STORY, nb), (1, STORY)], k * SLOTS + off * STORY)

                def t0v(off=0, nb=BSH):
                    return _mk_ap(t0[:], [(STORY, nb), (1, STORY)], off * STORY)

                def t0f(off, n):
                    return _mk_ap(t0[:], [(1, n)], off)

                ta_b = _mk_ap(t_tat[:], [(0, BSH), (1, STORY)])
                tc_bh = _mk_ap(t_tct[:], [(0, BSH // 2), (1, STORY)])
                u_b = _mk_ap(u[:], [(1, BSH), (0, STORY)])
                HB = SLOTS // 2  # 400

                for k in range(HOPS):
                    # t0 = (scale * S[k] + TA bcast) * u bcast
                    nc.vector.scalar_tensor_tensor(
                        t0v(), smv(k), scale, ta_b,
                        mybir.AluOpType.mult, mybir.AluOpType.add)
                    nc.vector.tensor_mul(t0v(), t0v(), u_b)
                    # partition-reduce -> logits [1, 16, 50] (two 400-wide psum banks)
                    for h in range(2):
                        pl = ppool.tile([1, HB], f32, tag="pl", space="PSUM")
                        nc.tensor.matmul(
                            pl[:], lhsT=ones_col[:], rhs=t0f(h * HB, HB),
                            start=True, stop=True)
                        nc.vector.tensor_copy(
                            _mk_ap(lg[:], [(1, HB)], h * HB), pl[:])
                    # softmax over story per batch, on partition 0
                    nc.vector.tensor_reduce(red[:], lg[:], mybir.AxisListType.X, mybir.AluOpType.max)
                    red_b = _mk_ap(red[:], [(1, BSH), (0, STORY)])
                    nc.vector.tensor_sub(lg[:], lg[:], red_b)
                    nc.scalar.activation(lg[:], lg[:], mybir.ActivationFunctionType.Exp)
                    nc.vector.tensor_reduce(red2[:], lg[:], mybir.AxisListType.X, mybir.AluOpType.add)
                    nc.vector.reciprocal(red2[:], red2[:])
                    red2_b = _mk_ap(red2[:], [(1, BSH), (0, STORY)])
                    nc.vector.tensor_mul(pe_sb[0:1, :, :], lg[:], red2_b)
                    # broadcast p to all partitions; then t0 = (SM[k+1] + TC bcast) * p
                    for h in range(2):
                        pb = ppool.tile([128, HB], f32, tag="pb", space="PSUM")
                        nc.tensor.matmul(
                            pb[:], lhsT=e0row[:],
                            rhs=_mk_ap(pe_sb[:], [(1, HB)], h * HB),
                            start=True, stop=True)
                        pb3 = _mk_ap(pb[:], [(STORY, BSH // 2), (1, STORY)])
                        nc.vector.scalar_tensor_tensor(
                            t0v(h * (BSH // 2), BSH // 2),
                            smv(k + 1, h * (BSH // 2), BSH // 2), scale, tc_bh,
                            mybir.AluOpType.mult, mybir.AluOpType.add)
                        nc.vector.tensor_mul(
                            t0v(h * (BSH // 2), BSH // 2),
                            t0v(h * (BSH // 2), BSH // 2), pb3)
                    # u += sum_s t0
                    nc.vector.tensor_reduce(red_u[:], t0v(), mybir.AxisListType.X, mybir.AluOpType.add)
                    nc.vector.tensor_add(u[:], u[:], red_u[:])

                # ---- projection: out[v, b] = sum_e A3[v, e] * u[e, b]
                u16 = hpool.tile([128, BSH], f16, tag="u16")
                nc.vector.tensor_copy(u16[:], u[:])
                ocache = hpool.tile([128, NVC, BSH], f32, tag="ocache")
                with tc.tile_pool(name="a3pool", bufs=3) as apool:
                    CPL = 32  # vocab chunks (of 128) per a3t load; 32*16 = 512 f32 = 1 PSUM bank
                    for blk in range(NVC // CPL + (1 if NVC % CPL else 0)):
                        n_in_blk = min(CPL, NVC - blk * CPL)
                        a3c = apool.tile([128, CPL * 128], f16, tag="a3c")
                        nc.sync.dma_start(
                            a3c[:, :n_in_blk * 128],
                            a3t[:, blk * CPL * 128: blk * CPL * 128 + n_in_blk * 128])
                        po = ppool.tile([128, CPL * BSH], f32, tag="po", space="PSUM")
                        for w in range(n_in_blk):
                            nc.tensor.matmul(
                                po[:, w * BSH:(w + 1) * BSH],
                                lhsT=a3c[:, w * 128:(w + 1) * 128],
                                rhs=u16[:], start=True, stop=True)
                        c0 = blk * CPL
                        nc.vector.tensor_copy(
                            ocache[:, c0:c0 + n_in_blk, :], po[:, :n_in_blk * BSH])
                nc.sync.dma_start(out[:], ocache[:])

    nc.compile()
    nc._gather_names = gather_names
    return nc


def _build_tuned(scale):
    """Two-pass build: learn scheduled SWDGE order, then align queue_num with
    Tile's DMASW-lane round-robin (lane n%8 must always see queue n%4)."""
    nc0 = _build(scale)
    sched = []
    for b in nc0.main_func.blocks:
        for i in b.instructions:
            if type(i).__name__ == "InstDMAGatherAnt":
                sched.append(i.name)
    return nc0  # multi-queue SWDGE corrupts concurrent transpose gathers on HW


def _prep_inputs(x, q, A, TA, TC):
    """Host-side marshalling: megatable, A3^T, per-core index lists."""
    x = np.asarray(x).astype(np.int64)
    q = np.asarray(q).astype(np.int64)
    A = np.asarray(A, dtype=np.float32)
    TA = np.asarray(TA, dtype=np.float32)
    TC = np.asarray(TC, dtype=np.float32)

    s = float(np.abs(A).max())
    scale = s / 32767.0
    Aq = np.round(A / s * 32767.0).astype(np.int16)
    # megarow v: [A0, A1, A2, A3] int16 (1024 B)
    mega = np.zeros((NROWS, 512), dtype=np.int16)
    for k in range(4):
        mega[:VOCAB, k * 128:(k + 1) * 128] = Aq[k]
    # row ZROW stays zero (hi-pass sentinel target); row 0 is zero already (padding row)

    a3t = np.zeros((128, VPAD), dtype=np.float16)
    a3t[:, :VOCAB] = A[3].astype(np.float16).T

    tat = np.ascontiguousarray(TA[0].T)  # [128, 50]
    tct = np.ascontiguousarray(TC[0].T)

    in_maps = []
    for c in range(NCORES):
        xs = x[c * BSH:(c + 1) * BSH].reshape(-1)        # [16000] slot-major
        lo = np.where(xs < SPLIT, xs, 0)
        hi = np.where(xs >= SPLIT, xs - SPLIT, HI_SENT)
        qs = q[c * BSH:(c + 1) * BSH]                     # [16, 20]
        qp = np.full((BSH, QPAD), -1, dtype=np.int64)
        qp[:, :QLEN] = qs
        qf = qp.reshape(-1)
        qlo = np.where((qf >= 0) & (qf < SPLIT), qf, 0)
        qhi = np.where(qf >= SPLIT, qf - SPLIT, HI_SENT)
        in_maps.append({
            "mega": mega, "a3t": a3t, "tat": tat, "tct": tct,
            "ilo": _wrap_idxs(lo), "ihi": _wrap_idxs(hi),
            "iqlo": _wrap_idxs(qlo), "iqhi": _wrap_idxs(qhi),
        })
    return in_maps, scale


def kernel(x, q, A, TA, TC):
    import os
    from concourse.bass_utils import run_bass_kernel_spmd

    in_maps, scale = _prep_inputs(x, q, A, TA, TC)
    if _cache.get("scale") != scale:
        _cache["nc"] = _build_tuned(scale)
        _cache["scale"] = scale
    nc = _cache["nc"]
    trace = bool(int(os.environ.get("MEMNN_TRACE", "0")))
    res = run_bass_kernel_spmd(nc, in_maps, list(range(NCORES)), trace=trace)
    if trace:
        _cache["exec_time_ns"] = res.exec_time_ns
        _cache["mean_exec_time_ns"] = res.mean_exec_time_ns
        _cache["results"] = res

    outs = []
    for c in range(NCORES):
        oc = res.results[c]["outp"]                       # [128, 391, 16]
        full = oc.transpose(1, 0, 2).reshape(VPAD, BSH)   # [50048, 16]
        outs.append(full[:VOCAB].T)                       # [16, 50000]
    return np.concatenate(outs, axis=0).astype(np.float32)
